# revision 1
# baseline (speedup 1.0000x reference)
"""Trainium2 Bass kernel for nn_CrossModalAttention (B=16384, GNN=512, TR=768, F=1024).

Math (seq_len==1 degenerate attention, see reference):
    gp = g @ Wg.T + bg                       [B, F]
    tp = t @ Wt.T + bt                       [B, F]
    ga = (tp @ Wv.T + bv) @ Wo.T + bo        (attention(g, t, t))
    ta = (gp @ Wv.T + bv) @ Wo.T + bo
    h  = gelu([ga, ta] @ W1.T + b1)
    out = h @ W2.T + b2 + gp + tp

The attention block is affine, so it folds into W1 on the host:
    M1 = W1[:, :F] @ Wo @ Wv   (multiplies tp)
    M2 = W1[:, F:] @ Wo @ Wv   (multiplies gp)
    c  = (W1[:, :F] + W1[:, F:]) @ (Wo @ bv + bo) + b1
    h  = gelu(M1 @ tp.T + M2 @ gp.T + c)     (transposed layout)

Device kernel works in transposed layout [feature, batch] so the matmul
contraction dim always lands on SBUF partitions; host transposes in/out.
Data parallel over 8 cores: each core owns 2048 batch rows.
"""

import sys

import numpy as np

for _p in ("/opt/trn_rl_repo", "/root/.axon_site/_ro/trn_rl_repo"):
    if _p not in sys.path:
        sys.path.append(_p)

import ml_dtypes

import concourse.bass as bass
import concourse.mybir as mybir
import concourse.tile as tile
from concourse.bass import ts
from concourse.bass_utils import run_bass_kernel_spmd

B = 16384
GNN = 512
TR = 768
F = 1024
N_CORES = 8
B_LOC = B // N_CORES  # 2048
P = 128

# Stage dtypes: AB = the gp/tp projections (dominant output terms),
# CD = the folded-attention/fusion branch (small contribution to output).
# "bf16x2" = hi/lo bf16 split of inputs+weights, 3 matmuls per K-tile
# (drops only the lo*lo term): ~1e-5 rel err at 3x bf16 cost.
AB_DT = "f32r"  # "f32r" | "bf16" | "f32" | "bf16x2"
CD_DT = "bf16"  # "bf16" | "f32r" | "f32"
NB = 512  # batch-column block per step
PSUM_BUFS = 8
IO_BUFS = 1
AF = mybir.ActivationFunctionType


def _np_dt(sdt):
    return ml_dtypes.bfloat16 if sdt == "bf16" else np.float32


def _mb_dt(sdt):
    return {
        "bf16": mybir.dt.bfloat16,
        "f32r": mybir.dt.float32r,
        "f32": mybir.dt.float32,
    }[sdt]


def _mm_cast(ap, sdt):
    """Bitcast a float32 AP to float32r for reduced-precision full-rate matmul."""
    if sdt == "f32r":
        return ap.bitcast(mybir.dt.float32r)
    return ap


_DMA_OPCODES = ("DMACopy", "DMATranspose", "EventSemaphore", "TriggeredCopy")


def _legalize_waits(bir: dict) -> dict:
    """Walrus on this stack accepts only ONE sync-wait per engine instruction
    ("Too many sync wait commands"). Hoist extra waits onto standalone
    EventSemaphore ops (what nc.<engine>.wait_ge emits) on the same engine."""
    ctr = 0

    def hoist(out, inst, w):
        nonlocal ctr
        ctr += 1
        out.append(
            {
                "debug": inst.get("debug", 0),
                "engine": inst["engine"],
                "ins": [],
                "outs": [],
                "name": f"I-lgw-{ctr}",
                "opcode": "EventSemaphore",
                "sync_info": {"on_update": [], "on_wait": [w]},
            }
        )

    for fn in bir["functions"]:
        for blk in fn["blocks"]:
            out = []
            for inst in blk["instructions"]:
                si = inst.get("sync_info")
                waits = (si.get("on_wait") or []) if si else []
                op = inst.get("opcode")
                if op == "EventSemaphore":
                    pass
                elif op in ("DMACopy", "DMATranspose", "TriggeredCopy"):
                    # keep one wait (prefer a queue DMA* sem) on the descriptor,
                    # hoist the rest onto the issuing sequencer
                    if len(waits) > 1:
                        keep = [w for w in waits if w["ant_name"].startswith("DMA")]
                        drop = [w for w in waits if not w["ant_name"].startswith("DMA")]
                        if not keep:
                            keep = [waits[-1]]
                            drop = waits[:-1]
                        while len(keep) > 1:
                            drop.append(keep.pop(0))
                        for w in drop:
                            hoist(out, inst, w)
                        si["on_wait"] = keep
                elif len(waits) > 1:
                    for w in waits[:-1]:
                        hoist(out, inst, w)
                    si["on_wait"] = waits[-1:]
                out.append(inst)
            blk["instructions"] = out
    return bir


def _attach_wait_legalizer(nc):
    import json as _json

    orig_fn = nc.to_json_bytes

    def _patched():
        bir = _json.loads(orig_fn())
        _legalize_waits(bir)
        return _json.dumps(bir).encode()

    nc.to_json_bytes = _patched


def build_module(repeat=1):
    nc = bass.Bass()
    f32 = mybir.dt.float32
    # tensors consumed by an fp32r matmul must themselves be declared fp32r
    # end-to-end (walrus birverifier "not rounded to FP32r" check)
    ab_io = _mb_dt(AB_DT)
    cd_io = _mb_dt(CD_DT)

    gT = nc.dram_tensor("gT", [GNN, B_LOC], ab_io, kind="ExternalInput")
    tT = nc.dram_tensor("tT", [TR, B_LOC], ab_io, kind="ExternalInput")
    wgT = nc.dram_tensor("wgT", [GNN, F], ab_io, kind="ExternalInput")
    wtT = nc.dram_tensor("wtT", [TR, F], ab_io, kind="ExternalInput")
    mcT = nc.dram_tensor("mcT", [2 * F, F], cd_io, kind="ExternalInput")
    w2T = nc.dram_tensor("w2T", [F, F], cd_io, kind="ExternalInput")
    bg = nc.dram_tensor("bg", [F], f32, kind="ExternalInput")
    bt = nc.dram_tensor("bt", [F], f32, kind="ExternalInput")
    cv = nc.dram_tensor("cv", [F], f32, kind="ExternalInput")
    b2 = nc.dram_tensor("b2", [F], f32, kind="ExternalInput")
    outT = nc.dram_tensor("outT", [F, B_LOC], f32, kind="ExternalOutput")

    KG = GNN // P  # 4
    KT = TR // P  # 6
    KF = F // P  # 8
    NBLK = B_LOC // NB

    g_ap = gT[:].rearrange("(k p) b -> p k b", p=P)
    t_ap = tT[:].rearrange("(k p) b -> p k b", p=P)
    out_ap = outT[:].rearrange("(k p) b -> p k b", p=P)

    with tile.TileContext(nc) as tc:
        with (
            tc.tile_pool(name="const", bufs=1) as const,
            tc.tile_pool(name="io", bufs=IO_BUFS) as io,
            tc.tile_pool(name="act", bufs=1) as act,
            tc.tile_pool(name="psum", bufs=PSUM_BUFS, space="PSUM") as psum,
        ):
            wg = const.tile([P, KG, F], _mb_dt(AB_DT))
            nc.sync.dma_start(out=wg, in_=wgT[:].rearrange("(k p) f -> p k f", p=P))
            wt = const.tile([P, KT, F], _mb_dt(AB_DT))
            nc.sync.dma_start(out=wt, in_=wtT[:].rearrange("(k p) f -> p k f", p=P))
            bg_t = const.tile([P, KF], f32)
            nc.sync.dma_start(out=bg_t, in_=bg[:].rearrange("(k p) -> p k", p=P))
            bt_t = const.tile([P, KF], f32)
            nc.sync.dma_start(out=bt_t, in_=bt[:].rearrange("(k p) -> p k", p=P))
            cv_t = const.tile([P, KF], f32)
            nc.sync.dma_start(out=cv_t, in_=cv[:].rearrange("(k p) -> p k", p=P))
            b2_t = const.tile([P, KF], f32)
            nc.sync.dma_start(out=b2_t, in_=b2[:].rearrange("(k p) -> p k", p=P))
            mc = const.tile([P, 2 * KF, F], _mb_dt(CD_DT))
            nc.sync.dma_start(out=mc, in_=mcT[:].rearrange("(k p) f -> p k f", p=P))
            w2 = const.tile([P, KF, F], _mb_dt(CD_DT))
            nc.sync.dma_start(out=w2, in_=w2T[:].rearrange("(k p) f -> p k f", p=P))

            for blk in [b for _ in range(repeat) for b in range(NBLK)]:
                bs = slice(blk * NB, (blk + 1) * NB)
                g_in = io.tile([P, KG, NB], wg.dtype, tag="g_in")
                nc.sync.dma_start(out=g_in, in_=g_ap[:, :, bs])
                t_in = io.tile([P, KT, NB], wt.dtype, tag="t_in")
                nc.sync.dma_start(out=t_in, in_=t_ap[:, :, bs])

                act_dt = mybir.dt.float32r if CD_DT == "f32r" else f32
                gp = act.tile([P, KF, NB], act_dt, tag="gp")
                tp = act.tile([P, KF, NB], act_dt, tag="tp")
                if CD_DT == "bf16":
                    gpb = act.tile([P, KF, NB], mybir.dt.bfloat16, tag="gpb")
                    tpb = act.tile([P, KF, NB], mybir.dt.bfloat16, tag="tpb")

                # A: gp = Wg @ g (+bg);  B: tp = Wt @ t (+bt)
                for w_t, x_in, y, yb, b_t, kk in (
                    (wg, g_in, gp, "gpb", bg_t, KG),
                    (wt, t_in, tp, "tpb", bt_t, KT),
                ):
                    for j in range(KF):
                        ps = psum.tile([P, NB], f32, tag="ps")
                        for k in range(kk):
                            nc.tensor.matmul(
                                ps,
                                _mm_cast(w_t[:, k, ts(j, P)], AB_DT),
                                _mm_cast(x_in[:, k, :], AB_DT),
                                start=(k == 0),
                                stop=(k == kk - 1),
                            )
                        nc.scalar.activation(y[:, j, :], ps, AF.Identity, bias=b_t[:, j : j + 1])
                        if CD_DT == "bf16":
                            dst = gpb if yb == "gpb" else tpb
                            nc.vector.tensor_copy(dst[:, j, :], y[:, j, :])

                # C: h = gelu(M2 @ gp + M1 @ tp + c)   (gp half first: ready earlier)
                rhs_g = gpb if CD_DT == "bf16" else gp
                rhs_t = tpb if CD_DT == "bf16" else tp
                h = act.tile([P, KF, NB], mc.dtype, tag="h")
                for j in range(KF):
                    ps = psum.tile([P, NB], f32, tag="ps")
                    for k in range(KF):
                        nc.tensor.matmul(
                            ps,
                            _mm_cast(mc[:, KF + k, ts(j, P)], CD_DT),
                            _mm_cast(rhs_g[:, k, :], CD_DT),
                            start=(k == 0),
                            stop=False,
                        )
                    for k in range(KF):
                        nc.tensor.matmul(
                            ps,
                            _mm_cast(mc[:, k, ts(j, P)], CD_DT),
                            _mm_cast(rhs_t[:, k, :], CD_DT),
                            start=False,
                            stop=(k == KF - 1),
                        )
                    nc.scalar.activation(h[:, j, :], ps, AF.Gelu, bias=cv_t[:, j : j + 1])

                # D: out = W2 @ h + b2 + gp + tp
                # epilogue all on DVE so the out DMA has a single-engine dep
                out_t = io.tile([P, KF, NB], f32, tag="out_t")
                for j in range(KF):
                    ps = psum.tile([P, NB], f32, tag="ps")
                    for k in range(KF):
                        nc.tensor.matmul(
                            ps,
                            _mm_cast(w2[:, k, ts(j, P)], CD_DT),
                            _mm_cast(h[:, k, :], CD_DT),
                            start=(k == 0),
                            stop=(k == KF - 1),
                        )
                    nc.vector.tensor_scalar_add(out_t[:, j, :], ps, b2_t[:, j : j + 1])
                    nc.vector.tensor_add(out_t[:, j, :], out_t[:, j, :], gp[:, j, :])
                    nc.vector.tensor_add(out_t[:, j, :], out_t[:, j, :], tp[:, j, :])
                nc.sync.dma_start(out=out_ap[:, :, bs], in_=out_t)

    _attach_wait_legalizer(nc)
    return nc


def prepare_inputs(gnn_features, transformer_features, Wg, bg, Wt, bt, Wv, bv, Wo, bo, W1, b1, W2, b2):
    """Host-side: fold the affine attention block into W1, transpose everything."""
    f64 = np.float64
    A = Wo.astype(f64) @ Wv.astype(f64)
    W1a = W1[:, :F].astype(f64)
    W1b = W1[:, F:].astype(f64)
    M1 = W1a @ A
    M2 = W1b @ A
    c = (W1a + W1b) @ (Wo.astype(f64) @ bv.astype(f64) + bo.astype(f64)) + b1.astype(f64)

    ab_np = _np_dt(AB_DT)
    cd_np = _np_dt(CD_DT)
    wgT = np.ascontiguousarray(Wg.T).astype(ab_np)
    wtT = np.ascontiguousarray(Wt.T).astype(ab_np)
    mcT = np.ascontiguousarray(np.concatenate([M1.T, M2.T], axis=0).astype(np.float32)).astype(cd_np)
    w2T = np.ascontiguousarray(W2.T).astype(cd_np)

    shared = {
        "wgT": wgT,
        "wtT": wtT,
        "mcT": mcT,
        "w2T": w2T,
        "bg": np.asarray(bg, np.float32),
        "bt": np.asarray(bt, np.float32),
        "cv": c.astype(np.float32),
        "b2": np.asarray(b2, np.float32),
    }
    in_maps = []
    for i in range(N_CORES):
        rows = slice(i * B_LOC, (i + 1) * B_LOC)
        in_maps.append(
            {
                "gT": np.ascontiguousarray(gnn_features[rows].T).astype(ab_np),
                "tT": np.ascontiguousarray(transformer_features[rows].T).astype(ab_np),
                **shared,
            }
        )
    return in_maps


def run(inputs, trace=False, **kw):
    nc = build_module()
    in_maps = prepare_inputs(**inputs)
    res = run_bass_kernel_spmd(nc, in_maps, core_ids=list(range(N_CORES)), trace=trace, **kw)
    out = np.concatenate([r["outT"].T for r in res.results], axis=0).astype(np.float32)
    return out, res


def kernel(**inputs) -> np.ndarray:
    out, _ = run(inputs, trace=False)
    return out



# revision 3
# speedup vs baseline: 2.6555x; 2.6555x over previous
"""Trainium2 Bass kernel for nn_CrossModalAttention (B=16384, GNN=512, TR=768, F=1024).

Math (seq_len==1 degenerate attention, see reference):
    gp = g @ Wg.T + bg                       [B, F]
    tp = t @ Wt.T + bt                       [B, F]
    ga = (tp @ Wv.T + bv) @ Wo.T + bo
    ta = (gp @ Wv.T + bv) @ Wo.T + bo
    h  = gelu([ga, ta] @ W1.T + b1)
    out = h @ W2.T + b2 + gp + tp

Everything upstream of the GELU is affine in the raw inputs, and the output
only ever needs gp+tp as a sum, so with x = [g; t] (K=1280):
    S = Ws @ x            Ws = [Wg | Wt]                      (= gp+tp-bias)
    u = G @ x + cu        G  = [M2@Wg | M1@Wt],  M1 = W1a@Wo@Wv, M2 = W1b@Wo@Wv
    h = gelu(u)
    out = W2 @ h + S + (bg+bt+b2)

All matmuls run as fp8-e4m3 DoubleRow (2 K-subtiles per instruction, 0.5
cycles/row). S needs more than fp8 precision, so it uses a 3-term hi/lo
split (drops only the lo*lo term): S = Wh@xh + Wh@xl + Wl@xh.  S-weights and
W2 both carry a 64x scale so S and D=W2@h accumulate into the SAME psum bank;
the single DVE epilogue adds the (pre-scaled) bias and the host divides the
bf16 output by 64 (exact).

Device layout is transposed [feature, batch]; data parallel over 8 cores
(2048 batch rows each).
"""

import sys

import numpy as np

for _p in ("/opt/trn_rl_repo", "/root/.axon_site/_ro/trn_rl_repo"):
    if _p not in sys.path:
        sys.path.append(_p)

import ml_dtypes

import concourse.bass as bass
import concourse.mybir as mybir
import concourse.tile as tile
from concourse.bass import ts
from concourse.bass_utils import run_bass_kernel_spmd

B = 16384
GNN = 512
TR = 768
F = 1024
K = GNN + TR  # 1280
N_CORES = 8
B_LOC = B // N_CORES  # 2048
P = 128
NB = 512  # batch-column block per step
NBLK = B_LOC // NB  # 4
K2 = K // (2 * P)  # 5 double-k-tiles for x-contractions
KF2 = F // (2 * P)  # 4 double-k-tiles for the h-contraction
KF = F // P  # 8 output row tiles

SCL_S = 64.0  # scale on Ws and W2 (shared psum scale)
SCL_U = 128.0  # scale on G

F8 = mybir.dt.float8e4
NP_F8 = mybir.dt.np(F8)
DR = mybir.MatmulPerfMode.DoubleRow
AF = mybir.ActivationFunctionType

PSUM_BUFS = 8
IO_BUFS = 2

_DMA_OPCODES = ("DMACopy", "DMATranspose", "EventSemaphore", "TriggeredCopy")


def _legalize_waits(bir: dict) -> dict:
    """Walrus on this stack accepts only ONE sync-wait per engine instruction
    ("Too many sync wait commands"). Hoist extra waits onto standalone
    EventSemaphore ops (what nc.<engine>.wait_ge emits) on the same engine."""
    ctr = 0

    def hoist(out, inst, w):
        nonlocal ctr
        ctr += 1
        out.append(
            {
                "debug": inst.get("debug", 0),
                "engine": inst["engine"],
                "ins": [],
                "outs": [],
                "name": f"I-lgw-{ctr}",
                "opcode": "EventSemaphore",
                "sync_info": {"on_update": [], "on_wait": [w]},
            }
        )

    for fn in bir["functions"]:
        for blk in fn["blocks"]:
            out = []
            for inst in blk["instructions"]:
                si = inst.get("sync_info")
                waits = (si.get("on_wait") or []) if si else []
                op = inst.get("opcode")
                if op == "EventSemaphore":
                    pass
                elif op in ("DMACopy", "DMATranspose", "TriggeredCopy"):
                    # keep one wait (prefer a queue DMA* sem) on the descriptor,
                    # hoist the rest onto the issuing sequencer
                    if len(waits) > 1:
                        keep = [w for w in waits if w["ant_name"].startswith("DMA")]
                        drop = [w for w in waits if not w["ant_name"].startswith("DMA")]
                        if not keep:
                            keep = [waits[-1]]
                            drop = waits[:-1]
                        while len(keep) > 1:
                            drop.append(keep.pop(0))
                        for w in drop:
                            hoist(out, inst, w)
                        si["on_wait"] = keep
                elif len(waits) > 1:
                    for w in waits[:-1]:
                        hoist(out, inst, w)
                    si["on_wait"] = waits[-1:]
                out.append(inst)
            blk["instructions"] = out
    return bir


def _attach_wait_legalizer(nc):
    import json as _json

    orig_fn = nc.to_json_bytes

    def _patched():
        bir = _json.loads(orig_fn())
        _legalize_waits(bir)
        return _json.dumps(bir).encode()

    nc.to_json_bytes = _patched


def build_module(repeat=1):
    nc = bass.Bass()
    f32 = mybir.dt.float32
    bf16 = mybir.dt.bfloat16

    xhi = nc.dram_tensor("xhi", [K, B_LOC], F8, kind="ExternalInput")
    xlo = nc.dram_tensor("xlo", [K, B_LOC], F8, kind="ExternalInput")
    wshi = nc.dram_tensor("wshi", [K, F], F8, kind="ExternalInput")
    wslo = nc.dram_tensor("wslo", [K, F], F8, kind="ExternalInput")
    g8 = nc.dram_tensor("g8", [K, F], F8, kind="ExternalInput")
    w28 = nc.dram_tensor("w28", [F, F], F8, kind="ExternalInput")
    # col 0..7: cu (gelu bias), col 8..15: 64*(bg+bt+b2), partition-major
    consts = nc.dram_tensor("consts", [P, 2 * KF], f32, kind="ExternalInput")
    outT = nc.dram_tensor("outT", [F, B_LOC], bf16, kind="ExternalOutput")

    xhi_ap = xhi[:].rearrange("(k two p) b -> p k two b", p=P, two=2)
    xlo_ap = xlo[:].rearrange("(k two p) b -> p k two b", p=P, two=2)
    out_ap = outT[:].rearrange("(k p) b -> p k b", p=P)

    with tile.TileContext(nc) as tc:
        with (
            tc.tile_pool(name="const", bufs=1) as const,
            tc.tile_pool(name="io", bufs=IO_BUFS) as io,
            tc.tile_pool(name="act", bufs=IO_BUFS) as act,
            tc.tile_pool(name="psum", bufs=PSUM_BUFS, space="PSUM") as psum,
        ):
            # weights: [P, k2, 2, F] so [:, k2, :, ts(j, P)] is a DoubleRow lhsT
            wg_t = const.tile([P, K2, 2, F], F8)
            ws_hi = const.tile([P, K2, 2, F], F8)
            ws_lo = const.tile([P, K2, 2, F], F8)
            w2_t = const.tile([P, KF2, 2, F], F8)
            cst = const.tile([P, 2 * KF], f32)
            nc.sync.dma_start(out=cst, in_=consts[:])
            # split weight DMAs per k2-chunk: fine-grained deps let the first
            # matmuls start before the full weight tensors have landed
            for k2 in range(K2):
                nc.sync.dma_start(
                    out=wg_t[:, k2],
                    in_=g8[ts(k2, 2 * P), :].rearrange("(two p) f -> p two f", p=P),
                )
            for k2 in range(K2):
                nc.sync.dma_start(
                    out=ws_hi[:, k2],
                    in_=wshi[ts(k2, 2 * P), :].rearrange("(two p) f -> p two f", p=P),
                )
            for k2 in range(K2):
                nc.sync.dma_start(
                    out=ws_lo[:, k2],
                    in_=wslo[ts(k2, 2 * P), :].rearrange("(two p) f -> p two f", p=P),
                )
            for k2 in range(KF2):
                nc.sync.dma_start(
                    out=w2_t[:, k2],
                    in_=w28[ts(k2, 2 * P), :].rearrange("(two p) f -> p two f", p=P),
                )

            for blk in [b for _ in range(repeat) for b in range(NBLK)]:
                bs = slice(blk * NB, (blk + 1) * NB)
                xh = io.tile([P, K2, 2, NB], F8, tag="xh")
                nc.sync.dma_start(out=xh, in_=xhi_ap[:, :, :, bs])
                xl = io.tile([P, K2, 2, NB], F8, tag="xl")
                nc.sync.dma_start(out=xl, in_=xlo_ap[:, :, :, bs])

                # U phase: h = gelu(G@x/SCL_U + cu), written directly as fp8
                h = act.tile([P, KF, NB], F8, tag="h")
                for j in range(KF):
                    ps = psum.tile([P, NB], f32, tag="ps")
                    for k2 in range(K2):
                        nc.tensor.matmul(
                            ps,
                            wg_t[:, k2, :, ts(j, P)],
                            xh[:, k2],
                            start=(k2 == 0),
                            stop=(k2 == K2 - 1),
                            perf_mode=DR,
                        )
                    nc.scalar.activation(
                        h[:, j, :], ps, AF.Gelu,
                        bias=cst[:, j : j + 1], scale=1.0 / SCL_U,
                    )

                # S+D phase into one psum bank: 64*(Ws@x + W2@h)
                out_t = io.tile([P, KF, NB], bf16, tag="out_t")
                for j in range(KF):
                    ps = psum.tile([P, NB], f32, tag="ps")
                    for k2 in range(K2):
                        nc.tensor.matmul(
                            ps, ws_hi[:, k2, :, ts(j, P)], xh[:, k2],
                            start=(k2 == 0), stop=False, perf_mode=DR,
                        )
                    for k2 in range(K2):
                        nc.tensor.matmul(
                            ps, ws_hi[:, k2, :, ts(j, P)], xl[:, k2],
                            start=False, stop=False, perf_mode=DR,
                        )
                    for k2 in range(K2):
                        nc.tensor.matmul(
                            ps, ws_lo[:, k2, :, ts(j, P)], xh[:, k2],
                            start=False, stop=False, perf_mode=DR,
                        )
                    for k2 in range(KF2):
                        nc.tensor.matmul(
                            ps, w2_t[:, k2, :, ts(j, P)], h[:, 2 * k2 : 2 * k2 + 2, :],
                            start=False, stop=(k2 == KF2 - 1), perf_mode=DR,
                        )
                    nc.vector.tensor_scalar_add(
                        out_t[:, j, :], ps, cst[:, KF + j : KF + j + 1]
                    )
                nc.sync.dma_start(out=out_ap[:, :, bs], in_=out_t)

    _attach_wait_legalizer(nc)
    return nc


def _q8(a):
    return np.asarray(a, np.float32).astype(NP_F8)


def prepare_inputs(gnn_features, transformer_features, Wg, bg, Wt, bt, Wv, bv, Wo, bo, W1, b1, W2, b2):
    """Host-side: fold the attention block + projections, fp8-quantize."""
    f64 = np.float64
    A = np.asarray(Wo, f64) @ np.asarray(Wv, f64)
    W1a = np.asarray(W1[:, :F], f64)
    W1b = np.asarray(W1[:, F:], f64)
    M1 = W1a @ A  # multiplies tp
    M2 = W1b @ A  # multiplies gp
    cu = (
        M1 @ np.asarray(bt, f64)
        + M2 @ np.asarray(bg, f64)
        + (W1a + W1b) @ (np.asarray(Wo, f64) @ np.asarray(bv, f64) + np.asarray(bo, f64))
        + np.asarray(b1, f64)
    )
    Ws = np.concatenate([np.asarray(Wg, f64), np.asarray(Wt, f64)], axis=1)  # [F, K]
    G = np.concatenate([M2 @ np.asarray(Wg, f64), M1 @ np.asarray(Wt, f64)], axis=1)
    btot = np.asarray(bg, f64) + np.asarray(bt, f64) + np.asarray(b2, f64)

    WsT32 = np.ascontiguousarray(Ws.T * SCL_S).astype(np.float32)  # [K, F]
    ws_hi = WsT32.astype(NP_F8)
    ws_lo = (WsT32 - ws_hi.astype(np.float32)).astype(NP_F8)
    g8 = np.ascontiguousarray(G.T * SCL_U).astype(np.float32).astype(NP_F8)
    w28 = np.ascontiguousarray(np.asarray(W2, f64).T * SCL_S).astype(np.float32).astype(NP_F8)

    # [P, 16] partition-major consts: col j = cu[j*?]... feature f = k*128+p
    cu_pk = np.ascontiguousarray(cu.astype(np.float32).reshape(KF, P).T)
    bt_pk = np.ascontiguousarray((btot * SCL_S).astype(np.float32).reshape(KF, P).T)
    consts = np.concatenate([cu_pk, bt_pk], axis=1)

    x = np.concatenate(
        [np.asarray(gnn_features, np.float32), np.asarray(transformer_features, np.float32)],
        axis=1,
    )  # [B, K]
    xh_full = x.astype(NP_F8)
    xl_full = (x - xh_full.astype(np.float32)).astype(NP_F8)

    shared = {
        "wshi": ws_hi,
        "wslo": ws_lo,
        "g8": g8,
        "w28": w28,
        "consts": consts,
    }
    in_maps = []
    for i in range(N_CORES):
        rows = slice(i * B_LOC, (i + 1) * B_LOC)
        in_maps.append(
            {
                "xhi": np.ascontiguousarray(xh_full[rows].T),
                "xlo": np.ascontiguousarray(xl_full[rows].T),
                **shared,
            }
        )
    return in_maps


def run(inputs, trace=False, **kw):
    nc = build_module()
    in_maps = prepare_inputs(**inputs)
    res = run_bass_kernel_spmd(nc, in_maps, core_ids=list(range(N_CORES)), trace=trace, **kw)
    out = np.concatenate(
        [r["outT"].T.astype(np.float32) for r in res.results], axis=0
    ) * (1.0 / SCL_S)
    return out, res


def kernel(**inputs) -> np.ndarray:
    out, _ = run(inputs, trace=False)
    return out


# revision 5
# speedup vs baseline: 3.1171x; 1.1738x over previous
"""Trainium2 Bass kernel for nn_CrossModalAttention (B=16384, GNN=512, TR=768, F=1024).

Math (seq_len==1 degenerate attention, see reference):
    gp = g @ Wg.T + bg                       [B, F]
    tp = t @ Wt.T + bt                       [B, F]
    ga = (tp @ Wv.T + bv) @ Wo.T + bo
    ta = (gp @ Wv.T + bv) @ Wo.T + bo
    h  = gelu([ga, ta] @ W1.T + b1)
    out = h @ W2.T + b2 + gp + tp

Everything upstream of the GELU is affine in the raw inputs, and the output
only ever needs gp+tp as a sum, so with x = [g; t] (K=1280):
    S = Ws @ x            Ws = [Wg | Wt]                      (= gp+tp-bias)
    u = G @ x + cu        G  = [M2@Wg | M1@Wt],  M1 = W1a@Wo@Wv, M2 = W1b@Wo@Wv
    h = gelu(u)
    out = W2 @ h + S + (bg+bt+b2)

All matmuls run as fp8-e4m3 DoubleRow (2 K-subtiles per instruction, 0.5
cycles/row). S needs more than fp8 precision, so it uses a 3-term hi/lo
split (drops only the lo*lo term): S = Wh@xh + Wh@xl + Wl@xh.  S-weights and
W2 both carry a 64x scale so S and D=W2@h accumulate into the SAME psum bank;
the single DVE epilogue adds the (pre-scaled) bias and the host divides the
bf16 output by 64 (exact).

Device layout is transposed [feature, batch]; data parallel over 8 cores
(2048 batch rows each).
"""

import sys

import numpy as np

for _p in ("/opt/trn_rl_repo", "/root/.axon_site/_ro/trn_rl_repo"):
    if _p not in sys.path:
        sys.path.append(_p)

import ml_dtypes

import concourse.bass as bass
import concourse.mybir as mybir
import concourse.tile as tile
from concourse.bass import ts
from concourse.bass_utils import run_bass_kernel_spmd

B = 16384
GNN = 512
TR = 768
F = 1024
K = GNN + TR  # 1280
N_CORES = 8
B_LOC = B // N_CORES  # 2048
P = 128
NB = 512  # batch-column block per step
NBLK = B_LOC // NB  # 4
K2 = K // (2 * P)  # 5 double-k-tiles for x-contractions
KF2 = F // (2 * P)  # 4 double-k-tiles for the h-contraction
KF = F // P  # 8 output row tiles

SCL_S = 64.0  # scale on Ws and W2 (shared psum scale)
SCL_U = 128.0  # scale on G

F8 = mybir.dt.float8e4
NP_F8 = mybir.dt.np(F8)
DR = mybir.MatmulPerfMode.DoubleRow
AF = mybir.ActivationFunctionType

PSUM_BUFS = 8
IO_BUFS = 2

_DMA_OPCODES = ("DMACopy", "DMATranspose", "EventSemaphore", "TriggeredCopy")


def _legalize_waits(bir: dict) -> dict:
    """Walrus on this stack accepts only ONE sync-wait per engine instruction
    ("Too many sync wait commands"). Hoist extra waits onto standalone
    EventSemaphore ops (what nc.<engine>.wait_ge emits) on the same engine."""
    ctr = 0

    def hoist(out, inst, w):
        nonlocal ctr
        ctr += 1
        out.append(
            {
                "debug": inst.get("debug", 0),
                "engine": inst["engine"],
                "ins": [],
                "outs": [],
                "name": f"I-lgw-{ctr}",
                "opcode": "EventSemaphore",
                "sync_info": {"on_update": [], "on_wait": [w]},
            }
        )

    for fn in bir["functions"]:
        for blk in fn["blocks"]:
            out = []
            for inst in blk["instructions"]:
                si = inst.get("sync_info")
                waits = (si.get("on_wait") or []) if si else []
                op = inst.get("opcode")
                if op == "EventSemaphore":
                    pass
                elif op in ("DMACopy", "DMATranspose", "TriggeredCopy"):
                    # keep one wait (prefer a queue DMA* sem) on the descriptor,
                    # hoist the rest onto the issuing sequencer
                    if len(waits) > 1:
                        keep = [w for w in waits if w["ant_name"].startswith("DMA")]
                        drop = [w for w in waits if not w["ant_name"].startswith("DMA")]
                        if not keep:
                            keep = [waits[-1]]
                            drop = waits[:-1]
                        while len(keep) > 1:
                            drop.append(keep.pop(0))
                        for w in drop:
                            hoist(out, inst, w)
                        si["on_wait"] = keep
                elif len(waits) > 1:
                    for w in waits[:-1]:
                        hoist(out, inst, w)
                    si["on_wait"] = waits[-1:]
                out.append(inst)
            blk["instructions"] = out
    return bir


def _attach_wait_legalizer(nc):
    import json as _json

    orig_fn = nc.to_json_bytes

    def _patched():
        bir = _json.loads(orig_fn())
        _legalize_waits(bir)
        return _json.dumps(bir).encode()

    nc.to_json_bytes = _patched


def build_module(repeat=1):
    nc = bass.Bass()
    f32 = mybir.dt.float32
    bf16 = mybir.dt.bfloat16

    xhi = nc.dram_tensor("xhi", [K, B_LOC], F8, kind="ExternalInput")
    xlo = nc.dram_tensor("xlo", [K, B_LOC], F8, kind="ExternalInput")
    wshi = nc.dram_tensor("wshi", [K, F], F8, kind="ExternalInput")
    wslo = nc.dram_tensor("wslo", [K, F], F8, kind="ExternalInput")
    g8 = nc.dram_tensor("g8", [K, F], F8, kind="ExternalInput")
    w28 = nc.dram_tensor("w28", [F, F], F8, kind="ExternalInput")
    # col 0..7: cu (gelu bias), col 8..15: 64*(bg+bt+b2), partition-major
    consts = nc.dram_tensor("consts", [P, 2 * KF], f32, kind="ExternalInput")
    outT = nc.dram_tensor("outT", [F, B_LOC], bf16, kind="ExternalOutput")

    xhi_ap = xhi[:].rearrange("(k two p) b -> p k two b", p=P, two=2)
    xlo_ap = xlo[:].rearrange("(k two p) b -> p k two b", p=P, two=2)
    out_ap = outT[:].rearrange("(k p) b -> p k b", p=P)

    with tile.TileContext(nc) as tc:
        with (
            tc.tile_pool(name="const", bufs=1) as const,
            tc.tile_pool(name="io", bufs=IO_BUFS) as io,
            tc.tile_pool(name="act", bufs=IO_BUFS) as act,
            tc.tile_pool(name="psum", bufs=PSUM_BUFS, space="PSUM") as psum,
        ):
            # weights: [P, k2, 2, F] so [:, k2, :, ts(j, P)] is a DoubleRow lhsT
            wg_t = const.tile([P, K2, 2, F], F8)
            ws_hi = const.tile([P, K2, 2, F], F8)
            ws_lo = const.tile([P, K2, 2, F], F8)
            w2_t = const.tile([P, KF2, 2, F], F8)
            cst = const.tile([P, 2 * KF], f32)

            def _ldw(dst, src, k2):
                nc.sync.dma_start(
                    out=dst[:, k2],
                    in_=src[ts(k2, 2 * P), :].rearrange("(two p) f -> p two f", p=P),
                )

            # DMA issue order = first-use order; DMAs serialize on the DMA
            # device, so block-0's x must not queue behind the big weights.
            x_tiles = {}
            x_tiles[0] = (
                io.tile([P, K2, 2, NB], F8, tag="xh", name="xh0"),
                io.tile([P, K2, 2, NB], F8, tag="xl", name="xl0"),
            )
            nc.sync.dma_start(out=x_tiles[0][0], in_=xhi_ap[:, :, :, 0:NB])
            for k2 in range(K2):
                _ldw(wg_t, g8, k2)
            nc.sync.dma_start(out=cst, in_=consts[:])
            nc.sync.dma_start(out=x_tiles[0][1], in_=xlo_ap[:, :, :, 0:NB])
            for k2 in range(K2):
                _ldw(ws_hi, wshi, k2)
            for k2 in range(K2):
                _ldw(ws_lo, wslo, k2)
            for k2 in range(KF2):
                _ldw(w2_t, w28, k2)

            for blk in [b for _ in range(repeat) for b in range(NBLK)]:
                bs = slice(blk * NB, (blk + 1) * NB)
                if blk not in x_tiles:
                    x_tiles[blk] = (
                        io.tile([P, K2, 2, NB], F8, tag="xh", name="xh_t"),
                        io.tile([P, K2, 2, NB], F8, tag="xl", name="xl_t"),
                    )
                    nc.sync.dma_start(out=x_tiles[blk][0], in_=xhi_ap[:, :, :, bs])
                    nc.sync.dma_start(out=x_tiles[blk][1], in_=xlo_ap[:, :, :, bs])
                xh, xl = x_tiles.pop(blk)

                # U phase: h = gelu(G@x/SCL_U + cu), written directly as fp8
                h = act.tile([P, KF, NB], F8, tag="h")
                for j in range(KF):
                    ps = psum.tile([P, NB], f32, tag="ps")
                    for k2 in range(K2):
                        nc.tensor.matmul(
                            ps,
                            wg_t[:, k2, :, ts(j, P)],
                            xh[:, k2],
                            start=(k2 == 0),
                            stop=(k2 == K2 - 1),
                            perf_mode=DR,
                        )
                    nc.scalar.activation(
                        h[:, j, :], ps, AF.Gelu,
                        bias=cst[:, j : j + 1], scale=1.0 / SCL_U,
                    )

                # S+D phase: 64*(Ws@x + W2@h), all 8 psum banks concurrently;
                # pass-ordered so later-arriving weights are needed later
                out_t = io.tile([P, KF, NB], bf16, tag="out_t")
                pss = [psum.tile([P, NB], f32, tag="ps", name=f"ps{j}") for j in range(KF)]
                for j in range(KF):
                    for k2 in range(K2):
                        nc.tensor.matmul(
                            pss[j], ws_hi[:, k2, :, ts(j, P)], xh[:, k2],
                            start=(k2 == 0), stop=False, perf_mode=DR,
                        )
                for j in range(KF):
                    for k2 in range(K2):
                        nc.tensor.matmul(
                            pss[j], ws_hi[:, k2, :, ts(j, P)], xl[:, k2],
                            start=False, stop=False, perf_mode=DR,
                        )
                for j in range(KF):
                    for k2 in range(K2):
                        nc.tensor.matmul(
                            pss[j], ws_lo[:, k2, :, ts(j, P)], xh[:, k2],
                            start=False, stop=False, perf_mode=DR,
                        )
                for j in range(KF):
                    for k2 in range(KF2):
                        nc.tensor.matmul(
                            pss[j], w2_t[:, k2, :, ts(j, P)], h[:, 2 * k2 : 2 * k2 + 2, :],
                            start=False, stop=(k2 == KF2 - 1), perf_mode=DR,
                        )
                    nc.vector.tensor_scalar_add(
                        out_t[:, j, :], pss[j], cst[:, KF + j : KF + j + 1]
                    )
                    nc.sync.dma_start(out=out_ap[:, j, bs], in_=out_t[:, j, :])

    _attach_wait_legalizer(nc)
    return nc


def _q8(a):
    return np.asarray(a, np.float32).astype(NP_F8)


def prepare_inputs(gnn_features, transformer_features, Wg, bg, Wt, bt, Wv, bv, Wo, bo, W1, b1, W2, b2):
    """Host-side: fold the attention block + projections, fp8-quantize."""
    f64 = np.float64
    A = np.asarray(Wo, f64) @ np.asarray(Wv, f64)
    W1a = np.asarray(W1[:, :F], f64)
    W1b = np.asarray(W1[:, F:], f64)
    M1 = W1a @ A  # multiplies tp
    M2 = W1b @ A  # multiplies gp
    cu = (
        M1 @ np.asarray(bt, f64)
        + M2 @ np.asarray(bg, f64)
        + (W1a + W1b) @ (np.asarray(Wo, f64) @ np.asarray(bv, f64) + np.asarray(bo, f64))
        + np.asarray(b1, f64)
    )
    Ws = np.concatenate([np.asarray(Wg, f64), np.asarray(Wt, f64)], axis=1)  # [F, K]
    G = np.concatenate([M2 @ np.asarray(Wg, f64), M1 @ np.asarray(Wt, f64)], axis=1)
    btot = np.asarray(bg, f64) + np.asarray(bt, f64) + np.asarray(b2, f64)

    WsT32 = np.ascontiguousarray(Ws.T * SCL_S).astype(np.float32)  # [K, F]
    ws_hi = WsT32.astype(NP_F8)
    ws_lo = (WsT32 - ws_hi.astype(np.float32)).astype(NP_F8)
    g8 = np.ascontiguousarray(G.T * SCL_U).astype(np.float32).astype(NP_F8)
    w28 = np.ascontiguousarray(np.asarray(W2, f64).T * SCL_S).astype(np.float32).astype(NP_F8)

    # [P, 16] partition-major consts: col j = cu[j*?]... feature f = k*128+p
    cu_pk = np.ascontiguousarray(cu.astype(np.float32).reshape(KF, P).T)
    bt_pk = np.ascontiguousarray((btot * SCL_S).astype(np.float32).reshape(KF, P).T)
    consts = np.concatenate([cu_pk, bt_pk], axis=1)

    x = np.concatenate(
        [np.asarray(gnn_features, np.float32), np.asarray(transformer_features, np.float32)],
        axis=1,
    )  # [B, K]
    xh_full = x.astype(NP_F8)
    xl_full = (x - xh_full.astype(np.float32)).astype(NP_F8)

    shared = {
        "wshi": ws_hi,
        "wslo": ws_lo,
        "g8": g8,
        "w28": w28,
        "consts": consts,
    }
    in_maps = []
    for i in range(N_CORES):
        rows = slice(i * B_LOC, (i + 1) * B_LOC)
        in_maps.append(
            {
                "xhi": np.ascontiguousarray(xh_full[rows].T),
                "xlo": np.ascontiguousarray(xl_full[rows].T),
                **shared,
            }
        )
    return in_maps


def run(inputs, trace=False, **kw):
    nc = build_module()
    in_maps = prepare_inputs(**inputs)
    res = run_bass_kernel_spmd(nc, in_maps, core_ids=list(range(N_CORES)), trace=trace, **kw)
    out = np.concatenate(
        [r["outT"].T.astype(np.float32) for r in res.results], axis=0
    ) * (1.0 / SCL_S)
    return out, res


def kernel(**inputs) -> np.ndarray:
    out, _ = run(inputs, trace=False)
    return out


# revision 8
# speedup vs baseline: 3.2107x; 1.0300x over previous
"""Trainium2 Bass kernel for nn_CrossModalAttention (B=16384, GNN=512, TR=768, F=1024).

Math (seq_len==1 degenerate attention, see reference):
    gp = g @ Wg.T + bg                       [B, F]
    tp = t @ Wt.T + bt                       [B, F]
    ga = (tp @ Wv.T + bv) @ Wo.T + bo
    ta = (gp @ Wv.T + bv) @ Wo.T + bo
    h  = gelu([ga, ta] @ W1.T + b1)
    out = h @ W2.T + b2 + gp + tp

Everything upstream of the GELU is affine in the raw inputs, and the output
only ever needs gp+tp as a sum, so with x = [g; t] (K=1280):
    S = Ws @ x            Ws = [Wg | Wt]                      (= gp+tp-bias)
    u = G @ x + cu        G  = [M2@Wg | M1@Wt],  M1 = W1a@Wo@Wv, M2 = W1b@Wo@Wv
    h = gelu(u)
    out = W2 @ h + S + (bg+bt+b2)

All matmuls run as fp8-e4m3 DoubleRow (2 K-subtiles per instruction, 0.5
cycles/row). S needs more than fp8 precision, so it uses a 3-term hi/lo
split (drops only the lo*lo term): S = Wh@xh + Wh@xl + Wl@xh.  S-weights and
W2 both carry a 64x scale so S and D=W2@h accumulate into the SAME psum bank;
the single DVE epilogue adds the (pre-scaled) bias and the host divides the
bf16 output by 64 (exact).

Device layout is transposed [feature, batch]; data parallel over 8 cores
(2048 batch rows each).
"""

import sys

import numpy as np

for _p in ("/opt/trn_rl_repo", "/root/.axon_site/_ro/trn_rl_repo"):
    if _p not in sys.path:
        sys.path.append(_p)

import ml_dtypes

import concourse.bass as bass
import concourse.mybir as mybir
import concourse.tile as tile
from concourse.bass import ts
from concourse.bass_utils import run_bass_kernel_spmd

B = 16384
GNN = 512
TR = 768
F = 1024
K = GNN + TR  # 1280
N_CORES = 8
B_LOC = B // N_CORES  # 2048
P = 128
NB = 512  # batch-column block per step
NBLK = B_LOC // NB  # 4
K2 = K // (2 * P)  # 5 double-k-tiles for x-contractions
KF2 = F // (2 * P)  # 4 double-k-tiles for the h-contraction
KF = F // P  # 8 output row tiles

SCL_S = 64.0  # scale on Ws and W2 (shared psum scale)
SCL_U = 128.0  # scale on G

F8 = mybir.dt.float8e4
NP_F8 = mybir.dt.np(F8)
DR = mybir.MatmulPerfMode.DoubleRow
AF = mybir.ActivationFunctionType

PSUM_BUFS = 8
IO_BUFS = 2

_DMA_OPCODES = ("DMACopy", "DMATranspose", "EventSemaphore", "TriggeredCopy")


def _legalize_waits(bir: dict) -> dict:
    """Walrus on this stack accepts only ONE sync-wait per engine instruction
    ("Too many sync wait commands"). Hoist extra waits onto standalone
    EventSemaphore ops (what nc.<engine>.wait_ge emits) on the same engine."""
    ctr = 0

    def hoist(out, inst, w):
        nonlocal ctr
        ctr += 1
        out.append(
            {
                "debug": inst.get("debug", 0),
                "engine": inst["engine"],
                "ins": [],
                "outs": [],
                "name": f"I-lgw-{ctr}",
                "opcode": "EventSemaphore",
                "sync_info": {"on_update": [], "on_wait": [w]},
            }
        )

    for fn in bir["functions"]:
        for blk in fn["blocks"]:
            out = []
            for inst in blk["instructions"]:
                si = inst.get("sync_info")
                waits = (si.get("on_wait") or []) if si else []
                op = inst.get("opcode")
                if op == "EventSemaphore":
                    pass
                elif op in ("DMACopy", "DMATranspose", "TriggeredCopy"):
                    # keep one wait (prefer a queue DMA* sem) on the descriptor,
                    # hoist the rest onto the issuing sequencer
                    if len(waits) > 1:
                        keep = [w for w in waits if w["ant_name"].startswith("DMA")]
                        drop = [w for w in waits if not w["ant_name"].startswith("DMA")]
                        if not keep:
                            keep = [waits[-1]]
                            drop = waits[:-1]
                        while len(keep) > 1:
                            drop.append(keep.pop(0))
                        for w in drop:
                            hoist(out, inst, w)
                        si["on_wait"] = keep
                elif len(waits) > 1:
                    for w in waits[:-1]:
                        hoist(out, inst, w)
                    si["on_wait"] = waits[-1:]
                out.append(inst)
            blk["instructions"] = out
    return bir


def _attach_wait_legalizer(nc):
    import json as _json

    orig_fn = nc.to_json_bytes

    def _patched():
        bir = _json.loads(orig_fn())
        _legalize_waits(bir)
        return _json.dumps(bir).encode()

    nc.to_json_bytes = _patched


def build_module(repeat=1):
    nc = bass.Bass()
    f32 = mybir.dt.float32
    bf16 = mybir.dt.bfloat16

    xhi = nc.dram_tensor("xhi", [K, B_LOC], F8, kind="ExternalInput")
    xlo = nc.dram_tensor("xlo", [K, B_LOC], F8, kind="ExternalInput")
    wshi = nc.dram_tensor("wshi", [K, F], F8, kind="ExternalInput")
    wslo = nc.dram_tensor("wslo", [K, F], F8, kind="ExternalInput")
    g8 = nc.dram_tensor("g8", [K, F], F8, kind="ExternalInput")
    w28 = nc.dram_tensor("w28", [F, F], F8, kind="ExternalInput")
    # col 0..7: cu (gelu bias), col 8..15: 64*(bg+bt+b2), partition-major
    consts = nc.dram_tensor("consts", [P, 2 * KF], f32, kind="ExternalInput")
    outT = nc.dram_tensor("outT", [F, B_LOC], bf16, kind="ExternalOutput")

    xhi_ap = xhi[:].rearrange("(k two p) b -> p k two b", p=P, two=2)
    xlo_ap = xlo[:].rearrange("(k two p) b -> p k two b", p=P, two=2)
    out_ap = outT[:].rearrange("(k p) b -> p k b", p=P)

    with tile.TileContext(nc) as tc:
        with (
            tc.tile_pool(name="const", bufs=1) as const,
            tc.tile_pool(name="io", bufs=IO_BUFS) as io,
            tc.tile_pool(name="act", bufs=IO_BUFS) as act,
            tc.tile_pool(name="psum", bufs=PSUM_BUFS, space="PSUM") as psum,
        ):
            # weights: [P, k2, 2, F] so [:, k2, :, ts(j, P)] is a DoubleRow lhsT
            wg_t = const.tile([P, K2, 2, F], F8)
            ws_hi = const.tile([P, K2, 2, F], F8)
            ws_lo = const.tile([P, K2, 2, F], F8)
            w2_t = const.tile([P, KF2, 2, F], F8)
            cst = const.tile([P, 2 * KF], f32)

            def _ldw(dst, src, k2):
                nc.sync.dma_start(
                    out=dst[:, k2],
                    in_=src[ts(k2, 2 * P), :].rearrange("(two p) f -> p two f", p=P),
                )

            # DMA issue order = first-use order; DMAs serialize on the DMA
            # device, so block-0's x must not queue behind the big weights.
            x_tiles = {}
            x_tiles[0] = (
                io.tile([P, K2, 2, NB], F8, tag="xh", name="xh0"),
                io.tile([P, K2, 2, NB], F8, tag="xl", name="xl0"),
            )
            nc.sync.dma_start(out=x_tiles[0][0][:, 0:3], in_=xhi_ap[:, 0:3, :, 0:NB])
            for k2 in range(3):
                _ldw(wg_t, g8, k2)
            nc.sync.dma_start(out=x_tiles[0][0][:, 3:K2], in_=xhi_ap[:, 3:K2, :, 0:NB])
            for k2 in range(3, K2):
                _ldw(wg_t, g8, k2)
            nc.sync.dma_start(out=cst, in_=consts[:])
            nc.sync.dma_start(out=x_tiles[0][1], in_=xlo_ap[:, :, :, 0:NB])
            for k2 in range(K2):
                _ldw(ws_hi, wshi, k2)
            for k2 in range(K2):
                _ldw(ws_lo, wslo, k2)
            for k2 in range(KF2):
                _ldw(w2_t, w28, k2)

            # warm up the PE pstate ramp during the initial DMA wait: dummy
            # DoubleRow matmuls on a memset tile (results never read as data)
            warm = const.tile([P, 2, NB], F8)
            wdrain = const.tile([P, 1], f32)
            nc.any.memset(warm, 0)
            wps = psum.tile([P, NB], f32, tag="ps", name="wps")
            for _ in range(20):
                nc.tensor.matmul(
                    wps, warm[:, :, 0:P], warm, start=True, stop=True, perf_mode=DR
                )
            nc.vector.tensor_copy(wdrain, wps[:, 0:1])

            for blk in [b for _ in range(repeat) for b in range(NBLK)]:
                bs = slice(blk * NB, (blk + 1) * NB)
                if blk not in x_tiles:
                    x_tiles[blk] = (
                        io.tile([P, K2, 2, NB], F8, tag="xh", name="xh_t"),
                        io.tile([P, K2, 2, NB], F8, tag="xl", name="xl_t"),
                    )
                    nc.sync.dma_start(out=x_tiles[blk][0], in_=xhi_ap[:, :, :, bs])
                    nc.sync.dma_start(out=x_tiles[blk][1], in_=xlo_ap[:, :, :, bs])
                xh, xl = x_tiles.pop(blk)

                # U phase: h = gelu(G@x/SCL_U + cu), written directly as fp8
                h = act.tile([P, KF, NB], F8, tag="h")
                for j in range(KF):
                    ps = psum.tile([P, NB], f32, tag="ps")
                    for k2 in range(K2):
                        nc.tensor.matmul(
                            ps,
                            wg_t[:, k2, :, ts(j, P)],
                            xh[:, k2],
                            start=(k2 == 0),
                            stop=(k2 == K2 - 1),
                            perf_mode=DR,
                        )
                    nc.scalar.activation(
                        h[:, j, :], ps, AF.Gelu,
                        bias=cst[:, j : j + 1], scale=1.0 / SCL_U,
                    )

                # S+D phase: 64*(Ws@x + W2@h), all 8 psum banks concurrently;
                # pass-ordered so later-arriving weights are needed later
                out_t = io.tile([P, KF, NB], bf16, tag="out_t")
                pss = [psum.tile([P, NB], f32, tag="ps", name=f"ps{j}") for j in range(KF)]
                for j in range(KF):
                    for k2 in range(K2):
                        nc.tensor.matmul(
                            pss[j], ws_hi[:, k2, :, ts(j, P)], xh[:, k2],
                            start=(k2 == 0), stop=False, perf_mode=DR,
                        )
                for j in range(KF):
                    for k2 in range(K2):
                        nc.tensor.matmul(
                            pss[j], ws_hi[:, k2, :, ts(j, P)], xl[:, k2],
                            start=False, stop=False, perf_mode=DR,
                        )
                for j in range(KF):
                    for k2 in range(K2):
                        nc.tensor.matmul(
                            pss[j], ws_lo[:, k2, :, ts(j, P)], xh[:, k2],
                            start=False, stop=False, perf_mode=DR,
                        )
                for j in range(KF):
                    for k2 in range(KF2):
                        nc.tensor.matmul(
                            pss[j], w2_t[:, k2, :, ts(j, P)], h[:, 2 * k2 : 2 * k2 + 2, :],
                            start=False, stop=(k2 == KF2 - 1), perf_mode=DR,
                        )
                    # alternate epilogue engine so neither falls behind PE
                    if j % 2 == 0:
                        nc.vector.tensor_scalar_add(
                            out_t[:, j, :], pss[j], cst[:, KF + j : KF + j + 1]
                        )
                    else:
                        nc.scalar.activation(
                            out_t[:, j, :], pss[j], AF.Identity,
                            bias=cst[:, KF + j : KF + j + 1],
                        )
                    if j % 2 == 1:
                        nc.sync.dma_start(
                            out=out_ap[:, j - 1 : j + 1, bs], in_=out_t[:, j - 1 : j + 1, :]
                        )

    _attach_wait_legalizer(nc)
    return nc


def _q8(a):
    return np.asarray(a, np.float32).astype(NP_F8)


def prepare_inputs(gnn_features, transformer_features, Wg, bg, Wt, bt, Wv, bv, Wo, bo, W1, b1, W2, b2):
    """Host-side: fold the attention block + projections, fp8-quantize."""
    f64 = np.float64
    A = np.asarray(Wo, f64) @ np.asarray(Wv, f64)
    W1a = np.asarray(W1[:, :F], f64)
    W1b = np.asarray(W1[:, F:], f64)
    M1 = W1a @ A  # multiplies tp
    M2 = W1b @ A  # multiplies gp
    cu = (
        M1 @ np.asarray(bt, f64)
        + M2 @ np.asarray(bg, f64)
        + (W1a + W1b) @ (np.asarray(Wo, f64) @ np.asarray(bv, f64) + np.asarray(bo, f64))
        + np.asarray(b1, f64)
    )
    Ws = np.concatenate([np.asarray(Wg, f64), np.asarray(Wt, f64)], axis=1)  # [F, K]
    G = np.concatenate([M2 @ np.asarray(Wg, f64), M1 @ np.asarray(Wt, f64)], axis=1)
    btot = np.asarray(bg, f64) + np.asarray(bt, f64) + np.asarray(b2, f64)

    WsT32 = np.ascontiguousarray(Ws.T * SCL_S).astype(np.float32)  # [K, F]
    ws_hi = WsT32.astype(NP_F8)
    ws_lo = (WsT32 - ws_hi.astype(np.float32)).astype(NP_F8)
    g8 = np.ascontiguousarray(G.T * SCL_U).astype(np.float32).astype(NP_F8)
    w28 = np.ascontiguousarray(np.asarray(W2, f64).T * SCL_S).astype(np.float32).astype(NP_F8)

    # [P, 16] partition-major consts: col j = cu[j*?]... feature f = k*128+p
    cu_pk = np.ascontiguousarray(cu.astype(np.float32).reshape(KF, P).T)
    bt_pk = np.ascontiguousarray((btot * SCL_S).astype(np.float32).reshape(KF, P).T)
    consts = np.concatenate([cu_pk, bt_pk], axis=1)

    x = np.concatenate(
        [np.asarray(gnn_features, np.float32), np.asarray(transformer_features, np.float32)],
        axis=1,
    )  # [B, K]
    xh_full = x.astype(NP_F8)
    xl_full = (x - xh_full.astype(np.float32)).astype(NP_F8)

    shared = {
        "wshi": ws_hi,
        "wslo": ws_lo,
        "g8": g8,
        "w28": w28,
        "consts": consts,
    }
    in_maps = []
    for i in range(N_CORES):
        rows = slice(i * B_LOC, (i + 1) * B_LOC)
        in_maps.append(
            {
                "xhi": np.ascontiguousarray(xh_full[rows].T),
                "xlo": np.ascontiguousarray(xl_full[rows].T),
                **shared,
            }
        )
    return in_maps


def run(inputs, trace=False, **kw):
    nc = build_module()
    in_maps = prepare_inputs(**inputs)
    res = run_bass_kernel_spmd(nc, in_maps, core_ids=list(range(N_CORES)), trace=trace, **kw)
    out = np.concatenate(
        [r["outT"].T.astype(np.float32) for r in res.results], axis=0
    ) * (1.0 / SCL_S)
    return out, res


def kernel(**inputs) -> np.ndarray:
    out, _ = run(inputs, trace=False)
    return out


# revision 10
# speedup vs baseline: 3.2604x; 1.0155x over previous
"""Trainium2 Bass kernel for nn_CrossModalAttention (B=16384, GNN=512, TR=768, F=1024).

Math (seq_len==1 degenerate attention, see reference):
    gp = g @ Wg.T + bg                       [B, F]
    tp = t @ Wt.T + bt                       [B, F]
    ga = (tp @ Wv.T + bv) @ Wo.T + bo
    ta = (gp @ Wv.T + bv) @ Wo.T + bo
    h  = gelu([ga, ta] @ W1.T + b1)
    out = h @ W2.T + b2 + gp + tp

Everything upstream of the GELU is affine in the raw inputs, and the output
only ever needs gp+tp as a sum, so with x = [g; t] (K=1280):
    S = Ws @ x            Ws = [Wg | Wt]                      (= gp+tp-bias)
    u = G @ x + cu        G  = [M2@Wg | M1@Wt],  M1 = W1a@Wo@Wv, M2 = W1b@Wo@Wv
    h = gelu(u)
    out = W2 @ h + S + (bg+bt+b2)

All matmuls run as fp8-e4m3 DoubleRow (2 K-subtiles per instruction, 0.5
cycles/row). S needs more than fp8 precision, so it uses a 3-term hi/lo
split (drops only the lo*lo term): S = Wh@xh + Wh@xl + Wl@xh.  S-weights and
W2 both carry a 64x scale so S and D=W2@h accumulate into the SAME psum bank;
the single DVE epilogue adds the (pre-scaled) bias and the host divides the
bf16 output by 64 (exact).

Device layout is transposed [feature, batch]; data parallel over 8 cores
(2048 batch rows each).
"""

import sys

import numpy as np

for _p in ("/opt/trn_rl_repo", "/root/.axon_site/_ro/trn_rl_repo"):
    if _p not in sys.path:
        sys.path.append(_p)

import ml_dtypes

import concourse.bass as bass
import concourse.mybir as mybir
import concourse.tile as tile
from concourse.bass import ts
from concourse.bass_utils import run_bass_kernel_spmd

B = 16384
GNN = 512
TR = 768
F = 1024
K = GNN + TR  # 1280
N_CORES = 8
B_LOC = B // N_CORES  # 2048
P = 128
NB = 512  # batch-column block per step
NBLK = B_LOC // NB  # 4
K2 = K // (2 * P)  # 5 double-k-tiles for x-contractions
KF2 = F // (2 * P)  # 4 double-k-tiles for the h-contraction
KF = F // P  # 8 output row tiles

SCL_S = 64.0  # scale on Ws and W2 (shared psum scale)
SCL_U = 128.0  # scale on G

F8 = mybir.dt.float8e4
NP_F8 = mybir.dt.np(F8)
DR = mybir.MatmulPerfMode.DoubleRow
AF = mybir.ActivationFunctionType

PSUM_BUFS = 8
IO_BUFS = 2

_DMA_OPCODES = ("DMACopy", "DMATranspose", "EventSemaphore", "TriggeredCopy")


def _legalize_waits(bir: dict) -> dict:
    """Walrus on this stack accepts only ONE sync-wait per engine instruction
    ("Too many sync wait commands"). Hoist extra waits onto standalone
    EventSemaphore ops (what nc.<engine>.wait_ge emits) on the same engine."""
    ctr = 0

    def hoist(out, inst, w):
        nonlocal ctr
        ctr += 1
        out.append(
            {
                "debug": inst.get("debug", 0),
                "engine": inst["engine"],
                "ins": [],
                "outs": [],
                "name": f"I-lgw-{ctr}",
                "opcode": "EventSemaphore",
                "sync_info": {"on_update": [], "on_wait": [w]},
            }
        )

    for fn in bir["functions"]:
        for blk in fn["blocks"]:
            out = []
            for inst in blk["instructions"]:
                si = inst.get("sync_info")
                waits = (si.get("on_wait") or []) if si else []
                op = inst.get("opcode")
                if op == "EventSemaphore":
                    pass
                elif op in ("DMACopy", "DMATranspose", "TriggeredCopy"):
                    # keep one wait (prefer a queue DMA* sem) on the descriptor,
                    # hoist the rest onto the issuing sequencer
                    if len(waits) > 1:
                        keep = [w for w in waits if w["ant_name"].startswith("DMA")]
                        drop = [w for w in waits if not w["ant_name"].startswith("DMA")]
                        if not keep:
                            keep = [waits[-1]]
                            drop = waits[:-1]
                        while len(keep) > 1:
                            drop.append(keep.pop(0))
                        for w in drop:
                            hoist(out, inst, w)
                        si["on_wait"] = keep
                elif len(waits) > 1:
                    for w in waits[:-1]:
                        hoist(out, inst, w)
                    si["on_wait"] = waits[-1:]
                out.append(inst)
            blk["instructions"] = out
    return bir


def _attach_wait_legalizer(nc):
    import json as _json

    orig_fn = nc.to_json_bytes

    def _patched():
        bir = _json.loads(orig_fn())
        _legalize_waits(bir)
        return _json.dumps(bir).encode()

    nc.to_json_bytes = _patched


def build_module(repeat=1):
    nc = bass.Bass()
    f32 = mybir.dt.float32
    bf16 = mybir.dt.bfloat16

    xhi = nc.dram_tensor("xhi", [K, B_LOC], F8, kind="ExternalInput")
    xlo = nc.dram_tensor("xlo", [K, B_LOC], F8, kind="ExternalInput")
    wshi = nc.dram_tensor("wshi", [K, F], F8, kind="ExternalInput")
    wslo = nc.dram_tensor("wslo", [K, F], F8, kind="ExternalInput")
    g8 = nc.dram_tensor("g8", [K, F], F8, kind="ExternalInput")
    w28 = nc.dram_tensor("w28", [F, F], F8, kind="ExternalInput")
    # col 0..7: cu (gelu bias), col 8..15: 64*(bg+bt+b2), partition-major
    consts = nc.dram_tensor("consts", [P, 2 * KF], f32, kind="ExternalInput")
    outT = nc.dram_tensor("outT", [F, B_LOC], bf16, kind="ExternalOutput")

    xhi_ap = xhi[:].rearrange("(k two p) b -> p k two b", p=P, two=2)
    xlo_ap = xlo[:].rearrange("(k two p) b -> p k two b", p=P, two=2)
    out_ap = outT[:].rearrange("(k p) b -> p k b", p=P)

    with tile.TileContext(nc) as tc:
        with (
            tc.tile_pool(name="const", bufs=1) as const,
            tc.tile_pool(name="io", bufs=IO_BUFS) as io,
            tc.tile_pool(name="act", bufs=IO_BUFS) as act,
            tc.tile_pool(name="psum", bufs=PSUM_BUFS, space="PSUM") as psum,
        ):
            # weights: [P, k2, 2, F] so [:, k2, :, ts(j, P)] is a DoubleRow lhsT
            wg_t = const.tile([P, K2, 2, F], F8)
            ws_hi = const.tile([P, K2, 2, F], F8)
            ws_lo = const.tile([P, K2, 2, F], F8)
            w2_t = const.tile([P, KF2, 2, F], F8)
            cst = const.tile([P, 2 * KF], f32)

            def _ldw(dst, src, k2):
                nc.sync.dma_start(
                    out=dst[:, k2],
                    in_=src[ts(k2, 2 * P), :].rearrange("(two p) f -> p two f", p=P),
                )

            # DMA issue order = first-use order; DMAs serialize on the DMA
            # device, so block-0's x must not queue behind the big weights.
            x_tiles = {}
            x_tiles[0] = (
                io.tile([P, K2, 2, NB], F8, tag="xh", name="xh0"),
                io.tile([P, K2, 2, NB], F8, tag="xl", name="xl0"),
            )
            nc.sync.dma_start(out=x_tiles[0][0][:, 0:3], in_=xhi_ap[:, 0:3, :, 0:NB])
            for k2 in range(3):
                _ldw(wg_t, g8, k2)
            nc.sync.dma_start(out=cst, in_=consts[:])
            nc.sync.dma_start(out=x_tiles[0][0][:, 3:K2], in_=xhi_ap[:, 3:K2, :, 0:NB])
            for k2 in range(3, K2):
                _ldw(wg_t, g8, k2)
            for k2 in range(K2):
                _ldw(ws_hi, wshi, k2)
            for k2 in range(K2):
                _ldw(ws_lo, wslo, k2)
            nc.sync.dma_start(out=x_tiles[0][1], in_=xlo_ap[:, :, :, 0:NB])
            for k2 in range(KF2):
                _ldw(w2_t, w28, k2)

            # warm up the PE pstate ramp during the initial DMA wait: dummy
            # DoubleRow matmuls on a memset tile (results never read as data)
            warm = const.tile([P, 2, NB], F8)
            wdrain = const.tile([P, 1], f32)
            nc.any.memset(warm, 0)
            wps = psum.tile([P, NB], f32, tag="ps", name="wps")
            for _ in range(13):
                nc.tensor.matmul(
                    wps, warm[:, :, 0:P], warm, start=True, stop=True, perf_mode=DR
                )
            nc.vector.tensor_copy(wdrain, wps[:, 0:1])

            for blk in [b for _ in range(repeat) for b in range(NBLK)]:
                bs = slice(blk * NB, (blk + 1) * NB)
                if blk not in x_tiles:
                    x_tiles[blk] = (
                        io.tile([P, K2, 2, NB], F8, tag="xh", name="xh_t"),
                        io.tile([P, K2, 2, NB], F8, tag="xl", name="xl_t"),
                    )
                    nc.sync.dma_start(out=x_tiles[blk][0], in_=xhi_ap[:, :, :, bs])
                    nc.sync.dma_start(out=x_tiles[blk][1], in_=xlo_ap[:, :, :, bs])
                xh, xl = x_tiles.pop(blk)

                # U phase: h = gelu(G@x/SCL_U + cu), written directly as fp8
                h = act.tile([P, KF, NB], F8, tag="h")
                for j in range(KF):
                    ps = psum.tile([P, NB], f32, tag="ps")
                    for k2 in range(K2):
                        nc.tensor.matmul(
                            ps,
                            wg_t[:, k2, :, ts(j, P)],
                            xh[:, k2],
                            start=(k2 == 0),
                            stop=(k2 == K2 - 1),
                            perf_mode=DR,
                        )
                    nc.scalar.activation(
                        h[:, j, :], ps, AF.Gelu,
                        bias=cst[:, j : j + 1], scale=1.0 / SCL_U,
                    )

                # S+D: 64*(Ws@x + W2@h) accumulated per psum bank j as
                # A: hi@xh, B: hi@xl, C: lo@xh, D: w2@h
                out_t = io.tile([P, KF, NB], bf16, tag="out_t")

                def _mm(j, w_t, x_t, k2, start, stop=False):
                    nc.tensor.matmul(
                        pss[j], w_t[:, k2, :, ts(j, P)], x_t[:, k2],
                        start=start, stop=stop, perf_mode=DR,
                    )

                def _fin(j):
                    for k2 in range(KF2):
                        nc.tensor.matmul(
                            pss[j], w2_t[:, k2, :, ts(j, P)],
                            h[:, 2 * k2 : 2 * k2 + 2, :],
                            start=False, stop=(k2 == KF2 - 1), perf_mode=DR,
                        )
                    # alternate epilogue engine so neither falls behind PE
                    if j % 2 == 0:
                        nc.vector.tensor_scalar_add(
                            out_t[:, j, :], pss[j], cst[:, KF + j : KF + j + 1]
                        )
                    else:
                        nc.scalar.activation(
                            out_t[:, j, :], pss[j], AF.Identity,
                            bias=cst[:, KF + j : KF + j + 1],
                        )
                    nc.sync.dma_start(out=out_ap[:, j, bs], in_=out_t[:, j, :])

                pss = [psum.tile([P, NB], f32, tag="ps", name=f"ps{j}") for j in range(KF)]
                if blk == 0:
                    # weights still streaming: k2-major passes in DMA-arrival
                    # order (wshi, wslo, xl, w28) so PE never outruns the DMA
                    for k2 in range(K2):
                        for j in range(KF):
                            _mm(j, ws_hi, xh, k2, start=(k2 == 0))
                    for k2 in range(K2):
                        for j in range(KF):
                            _mm(j, ws_lo, xh, k2, start=False)
                    for j in range(KF):
                        for k2 in range(K2):
                            _mm(j, ws_hi, xl, k2, start=False)
                    for j in range(KF):
                        _fin(j)
                else:
                    # weights resident: fused per-j so epilogues+out-DMAs
                    # stream across the whole phase (no tail bunching)
                    for j in range(KF):
                        for k2 in range(K2):
                            _mm(j, ws_hi, xh, k2, start=(k2 == 0))
                        for k2 in range(K2):
                            _mm(j, ws_hi, xl, k2, start=False)
                        for k2 in range(K2):
                            _mm(j, ws_lo, xh, k2, start=False)
                        _fin(j)

    _attach_wait_legalizer(nc)
    return nc


def _q8(a):
    return np.asarray(a, np.float32).astype(NP_F8)


def prepare_inputs(gnn_features, transformer_features, Wg, bg, Wt, bt, Wv, bv, Wo, bo, W1, b1, W2, b2):
    """Host-side: fold the attention block + projections, fp8-quantize."""
    f64 = np.float64
    A = np.asarray(Wo, f64) @ np.asarray(Wv, f64)
    W1a = np.asarray(W1[:, :F], f64)
    W1b = np.asarray(W1[:, F:], f64)
    M1 = W1a @ A  # multiplies tp
    M2 = W1b @ A  # multiplies gp
    cu = (
        M1 @ np.asarray(bt, f64)
        + M2 @ np.asarray(bg, f64)
        + (W1a + W1b) @ (np.asarray(Wo, f64) @ np.asarray(bv, f64) + np.asarray(bo, f64))
        + np.asarray(b1, f64)
    )
    Ws = np.concatenate([np.asarray(Wg, f64), np.asarray(Wt, f64)], axis=1)  # [F, K]
    G = np.concatenate([M2 @ np.asarray(Wg, f64), M1 @ np.asarray(Wt, f64)], axis=1)
    btot = np.asarray(bg, f64) + np.asarray(bt, f64) + np.asarray(b2, f64)

    WsT32 = np.ascontiguousarray(Ws.T * SCL_S).astype(np.float32)  # [K, F]
    ws_hi = WsT32.astype(NP_F8)
    ws_lo = (WsT32 - ws_hi.astype(np.float32)).astype(NP_F8)
    g8 = np.ascontiguousarray(G.T * SCL_U).astype(np.float32).astype(NP_F8)
    w28 = np.ascontiguousarray(np.asarray(W2, f64).T * SCL_S).astype(np.float32).astype(NP_F8)

    # [P, 16] partition-major consts: col j = cu[j*?]... feature f = k*128+p
    cu_pk = np.ascontiguousarray(cu.astype(np.float32).reshape(KF, P).T)
    bt_pk = np.ascontiguousarray((btot * SCL_S).astype(np.float32).reshape(KF, P).T)
    consts = np.concatenate([cu_pk, bt_pk], axis=1)

    x = np.concatenate(
        [np.asarray(gnn_features, np.float32), np.asarray(transformer_features, np.float32)],
        axis=1,
    )  # [B, K]
    xh_full = x.astype(NP_F8)
    xl_full = (x - xh_full.astype(np.float32)).astype(NP_F8)

    shared = {
        "wshi": ws_hi,
        "wslo": ws_lo,
        "g8": g8,
        "w28": w28,
        "consts": consts,
    }
    in_maps = []
    for i in range(N_CORES):
        rows = slice(i * B_LOC, (i + 1) * B_LOC)
        in_maps.append(
            {
                "xhi": np.ascontiguousarray(xh_full[rows].T),
                "xlo": np.ascontiguousarray(xl_full[rows].T),
                **shared,
            }
        )
    return in_maps


def run(inputs, trace=False, **kw):
    nc = build_module()
    in_maps = prepare_inputs(**inputs)
    res = run_bass_kernel_spmd(nc, in_maps, core_ids=list(range(N_CORES)), trace=trace, **kw)
    out = np.concatenate(
        [r["outT"].T.astype(np.float32) for r in res.results], axis=0
    ) * (1.0 / SCL_S)
    return out, res


def kernel(**inputs) -> np.ndarray:
    out, _ = run(inputs, trace=False)
    return out


# revision 14
# speedup vs baseline: 3.5018x; 1.0740x over previous
"""Trainium2 Bass kernel for nn_CrossModalAttention (B=16384, GNN=512, TR=768, F=1024).

Math (seq_len==1 degenerate attention, see reference):
    gp = g @ Wg.T + bg                       [B, F]
    tp = t @ Wt.T + bt                       [B, F]
    ga = (tp @ Wv.T + bv) @ Wo.T + bo
    ta = (gp @ Wv.T + bv) @ Wo.T + bo
    h  = gelu([ga, ta] @ W1.T + b1)
    out = h @ W2.T + b2 + gp + tp

Everything upstream of the GELU is affine in the raw inputs, and the output
only ever needs gp+tp as a sum, so with x = [g; t] (K=1280):
    S = Ws @ x            Ws = [Wg | Wt]                      (= gp+tp-bias)
    u = G @ x + cu        G  = [M2@Wg | M1@Wt],  M1 = W1a@Wo@Wv, M2 = W1b@Wo@Wv
    h = gelu(u)
    out = W2 @ h + S + (bg+bt+b2)

All matmuls run as fp8-e4m3 DoubleRow (2 K-subtiles per instruction, 0.5
cycles/row). S needs more than fp8 precision, so it uses a 3-term hi/lo
split (drops only the lo*lo term): S = Wh@xh + Wh@xl + Wl@xh.  S-weights and
W2 both carry a 64x scale so S and D=W2@h accumulate into the SAME psum bank;
the single DVE epilogue adds the (pre-scaled) bias and the host divides the
bf16 output by 64 (exact).

Device layout is transposed [feature, batch]; data parallel over 8 cores
(2048 batch rows each).
"""

import sys

import numpy as np

for _p in ("/opt/trn_rl_repo", "/root/.axon_site/_ro/trn_rl_repo"):
    if _p not in sys.path:
        sys.path.append(_p)

import ml_dtypes

import concourse.bass as bass
import concourse.mybir as mybir
import concourse.tile as tile
from concourse.bass import ts
from concourse.bass_utils import run_bass_kernel_spmd

B = 16384
GNN = 512
TR = 768
F = 1024
K = GNN + TR  # 1280
N_CORES = 8
B_LOC = B // N_CORES  # 2048
P = 128
NB = 512  # batch-column block per step
NBLK = B_LOC // NB  # 4
K2 = K // (2 * P)  # 5 double-k-tiles for x-contractions
KW = 3  # lo-weight correction applied to first KW of K2 pairs (precision dial)
KF2 = F // (2 * P)  # 4 double-k-tiles for the h-contraction
KF = F // P  # 8 output row tiles

SCL_S = 64.0  # scale on Ws and W2 (shared psum scale)
SCL_U = 128.0  # scale on G

F8 = mybir.dt.float8e4
NP_F8 = mybir.dt.np(F8)
DR = mybir.MatmulPerfMode.DoubleRow
AF = mybir.ActivationFunctionType

PSUM_BUFS = 8
IO_BUFS = 2

_DMA_OPCODES = ("DMACopy", "DMATranspose", "EventSemaphore", "TriggeredCopy")


def _legalize_waits(bir: dict) -> dict:
    """Walrus on this stack accepts only ONE sync-wait per engine instruction
    ("Too many sync wait commands"). Hoist extra waits onto standalone
    EventSemaphore ops (what nc.<engine>.wait_ge emits) on the same engine."""
    ctr = 0

    def hoist(out, inst, w):
        nonlocal ctr
        ctr += 1
        out.append(
            {
                "debug": inst.get("debug", 0),
                "engine": inst["engine"],
                "ins": [],
                "outs": [],
                "name": f"I-lgw-{ctr}",
                "opcode": "EventSemaphore",
                "sync_info": {"on_update": [], "on_wait": [w]},
            }
        )

    for fn in bir["functions"]:
        for blk in fn["blocks"]:
            out = []
            for inst in blk["instructions"]:
                si = inst.get("sync_info")
                waits = (si.get("on_wait") or []) if si else []
                op = inst.get("opcode")
                if op == "EventSemaphore":
                    pass
                elif op in ("DMACopy", "DMATranspose", "TriggeredCopy"):
                    # keep one wait (prefer a queue DMA* sem) on the descriptor,
                    # hoist the rest onto the issuing sequencer
                    if len(waits) > 1:
                        keep = [w for w in waits if w["ant_name"].startswith("DMA")]
                        drop = [w for w in waits if not w["ant_name"].startswith("DMA")]
                        if not keep:
                            keep = [waits[-1]]
                            drop = waits[:-1]
                        while len(keep) > 1:
                            drop.append(keep.pop(0))
                        for w in drop:
                            hoist(out, inst, w)
                        si["on_wait"] = keep
                elif len(waits) > 1:
                    for w in waits[:-1]:
                        hoist(out, inst, w)
                    si["on_wait"] = waits[-1:]
                out.append(inst)
            blk["instructions"] = out
    return bir


def _attach_wait_legalizer(nc):
    import json as _json

    orig_fn = nc.to_json_bytes

    def _patched():
        bir = _json.loads(orig_fn())
        _legalize_waits(bir)
        return _json.dumps(bir).encode()

    nc.to_json_bytes = _patched


def build_module(repeat=1):
    nc = bass.Bass()
    f32 = mybir.dt.float32
    bf16 = mybir.dt.bfloat16

    xhi = nc.dram_tensor("xhi", [K, B_LOC], F8, kind="ExternalInput")
    xlo = nc.dram_tensor("xlo", [K, B_LOC], F8, kind="ExternalInput")
    wshi = nc.dram_tensor("wshi", [K, F], F8, kind="ExternalInput")
    wslo = nc.dram_tensor("wslo", [KW * 2 * P, F], F8, kind="ExternalInput")
    g8 = nc.dram_tensor("g8", [K, F], F8, kind="ExternalInput")
    w28 = nc.dram_tensor("w28", [F, F], F8, kind="ExternalInput")
    # col 0..7: cu (gelu bias), col 8..15: 64*(bg+bt+b2), partition-major
    consts = nc.dram_tensor("consts", [P, 2 * KF], f32, kind="ExternalInput")
    outT = nc.dram_tensor("outT", [F, B_LOC], bf16, kind="ExternalOutput")

    xhi_ap = xhi[:].rearrange("(k two p) b -> p k two b", p=P, two=2)
    xlo_ap = xlo[:].rearrange("(k two p) b -> p k two b", p=P, two=2)
    out_ap = outT[:].rearrange("(k p) b -> p k b", p=P)

    with tile.TileContext(nc) as tc:
        with (
            tc.tile_pool(name="const", bufs=1) as const,
            tc.tile_pool(name="io", bufs=IO_BUFS) as io,
            tc.tile_pool(name="act", bufs=IO_BUFS) as act,
            tc.tile_pool(name="psum", bufs=PSUM_BUFS, space="PSUM") as psum,
        ):
            # weights: [P, k2, 2, F] so [:, k2, :, ts(j, P)] is a DoubleRow lhsT
            wg_t = const.tile([P, K2, 2, F], F8)
            ws_hi = const.tile([P, K2, 2, F], F8)
            ws_lo = const.tile([P, KW, 2, F], F8)
            w2_t = const.tile([P, KF2, 2, F], F8)
            cst = const.tile([P, 2 * KF], f32)

            def _ldw(dst, src, k2):
                nc.sync.dma_start(
                    out=dst[:, k2],
                    in_=src[ts(k2, 2 * P), :].rearrange("(two p) f -> p two f", p=P),
                )

            # DMA issue order = first-use order; DMAs serialize on the DMA
            # device, so block-0's x must not queue behind the big weights.
            x_tiles = {}
            x_tiles[0] = (
                io.tile([P, K2, 2, NB], F8, tag="xh", name="xh0"),
                io.tile([P, K2, 2, NB], F8, tag="xl", name="xl0"),
            )
            nc.sync.dma_start(out=x_tiles[0][0][:, 0:3], in_=xhi_ap[:, 0:3, :, 0:NB])
            for k2 in range(3):
                _ldw(wg_t, g8, k2)
            nc.sync.dma_start(out=cst, in_=consts[:])
            nc.sync.dma_start(out=x_tiles[0][0][:, 3:K2], in_=xhi_ap[:, 3:K2, :, 0:NB])
            for k2 in range(3, K2):
                _ldw(wg_t, g8, k2)
            for k2 in range(K2):
                _ldw(ws_hi, wshi, k2)
            for k2 in range(KW):
                _ldw(ws_lo, wslo, k2)
            nc.sync.dma_start(out=x_tiles[0][1], in_=xlo_ap[:, :, :, 0:NB])
            for k2 in range(KF2):
                _ldw(w2_t, w28, k2)

            # warm up the PE pstate ramp during the initial DMA wait: dummy
            # DoubleRow matmuls on a memset tile (results never read as data)
            warm = const.tile([P, 2, NB], F8)
            wdrain = const.tile([P, 1], f32)
            nc.vector.memset(warm, 0)
            wps = psum.tile([P, NB], f32, tag="ps", name="wps")
            for _ in range(15):
                nc.tensor.matmul(
                    wps, warm[:, :, 0:P], warm, start=True, stop=True, perf_mode=DR
                )
            nc.vector.tensor_copy(wdrain, wps[:, 0:1])

            for blk in [b for _ in range(repeat) for b in range(NBLK)]:
                bs = slice(blk * NB, (blk + 1) * NB)
                if blk not in x_tiles:
                    x_tiles[blk] = (
                        io.tile([P, K2, 2, NB], F8, tag="xh", name="xh_t"),
                        io.tile([P, K2, 2, NB], F8, tag="xl", name="xl_t"),
                    )
                    nc.sync.dma_start(out=x_tiles[blk][0], in_=xhi_ap[:, :, :, bs])
                    nc.sync.dma_start(out=x_tiles[blk][1], in_=xlo_ap[:, :, :, bs])
                xh, xl = x_tiles.pop(blk)

                # U phase: h = gelu(G@x/SCL_U + cu), written directly as fp8
                h = act.tile([P, KF, NB], F8, tag="h")
                for j in range(KF):
                    ps = psum.tile([P, NB], f32, tag="ps")
                    for k2 in range(K2):
                        nc.tensor.matmul(
                            ps,
                            wg_t[:, k2, :, ts(j, P)],
                            xh[:, k2],
                            start=(k2 == 0),
                            stop=(k2 == K2 - 1),
                            perf_mode=DR,
                        )
                    nc.scalar.activation(
                        h[:, j, :], ps, AF.Gelu,
                        bias=cst[:, j : j + 1], scale=1.0 / SCL_U,
                    )

                # S+D: 64*(Ws@x + W2@h) accumulated per psum bank j as
                # A: hi@xh, B: hi@xl, C: lo@xh, D: w2@h
                out_t = io.tile([P, KF, NB], bf16, tag="out_t")

                full = slice(0, NB)

                def _mm(j, w_t, x_t, k2, start, stop=False, cs=full):
                    nc.tensor.matmul(
                        pss[j][:, cs], w_t[:, k2, :, ts(j, P)], x_t[:, k2, :, cs],
                        start=start, stop=stop, perf_mode=DR,
                    )

                def _fin(j, cs=full):
                    for k2 in range(KF2):
                        nc.tensor.matmul(
                            pss[j][:, cs], w2_t[:, k2, :, ts(j, P)],
                            h[:, 2 * k2 : 2 * k2 + 2, cs],
                            start=False, stop=(k2 == KF2 - 1), perf_mode=DR,
                        )
                    # alternate epilogue engine so neither falls behind PE
                    if j % 2 == 0:
                        nc.vector.tensor_scalar_add(
                            out_t[:, j, cs], pss[j][:, cs], cst[:, KF + j : KF + j + 1]
                        )
                    else:
                        nc.scalar.activation(
                            out_t[:, j, cs], pss[j][:, cs], AF.Identity,
                            bias=cst[:, KF + j : KF + j + 1],
                        )
                    obs = slice(blk * NB + cs.start, blk * NB + cs.stop)
                    nc.sync.dma_start(out=out_ap[:, j, obs], in_=out_t[:, j, cs])

                pss = [psum.tile([P, NB], f32, tag="ps", name=f"ps{j}") for j in range(KF)]
                if blk == 0:
                    # weights still streaming: k2-major passes in DMA-arrival
                    # order (wshi, wslo, xl, w28) so PE never outruns the DMA
                    for k2 in range(K2):
                        for j in range(KF):
                            _mm(j, ws_hi, xh, k2, start=(k2 == 0))
                    for k2 in range(KW):
                        for j in range(KF):
                            _mm(j, ws_lo, xh, k2, start=False)
                    for j in range(KF):
                        for k2 in range(K2):
                            _mm(j, ws_hi, xl, k2, start=False)
                    for j in range(KF):
                        _fin(j)
                else:
                    # weights resident: fused per-j so epilogues+out-DMAs
                    # stream across the whole phase (no tail bunching)
                    for j in range(KF):
                        last = blk == NBLK - 1 and j == KF - 1
                        # very last output: process in column halves so the
                        # final epilogue+DMA chain after the last matmul is
                        # half-sized (it is pure tail latency)
                        halves = [slice(0, NB // 2), slice(NB // 2, NB)] if last else [slice(0, NB)]
                        for cs in halves:
                            for k2 in range(K2):
                                _mm(j, ws_hi, xh, k2, start=(k2 == 0), cs=cs)
                            for k2 in range(K2):
                                _mm(j, ws_hi, xl, k2, start=False, cs=cs)
                            for k2 in range(KW):
                                _mm(j, ws_lo, xh, k2, start=False, cs=cs)
                            _fin(j, cs=cs)

    _attach_wait_legalizer(nc)
    return nc


def _q8(a):
    return np.asarray(a, np.float32).astype(NP_F8)


def prepare_inputs(gnn_features, transformer_features, Wg, bg, Wt, bt, Wv, bv, Wo, bo, W1, b1, W2, b2):
    """Host-side: fold the attention block + projections, fp8-quantize."""
    f64 = np.float64
    A = np.asarray(Wo, f64) @ np.asarray(Wv, f64)
    W1a = np.asarray(W1[:, :F], f64)
    W1b = np.asarray(W1[:, F:], f64)
    M1 = W1a @ A  # multiplies tp
    M2 = W1b @ A  # multiplies gp
    cu = (
        M1 @ np.asarray(bt, f64)
        + M2 @ np.asarray(bg, f64)
        + (W1a + W1b) @ (np.asarray(Wo, f64) @ np.asarray(bv, f64) + np.asarray(bo, f64))
        + np.asarray(b1, f64)
    )
    Ws = np.concatenate([np.asarray(Wg, f64), np.asarray(Wt, f64)], axis=1)  # [F, K]
    G = np.concatenate([M2 @ np.asarray(Wg, f64), M1 @ np.asarray(Wt, f64)], axis=1)
    btot = np.asarray(bg, f64) + np.asarray(bt, f64) + np.asarray(b2, f64)

    WsT32 = np.ascontiguousarray(Ws.T * SCL_S).astype(np.float32)  # [K, F]
    ws_hi = WsT32.astype(NP_F8)
    ws_lo = (WsT32 - ws_hi.astype(np.float32)).astype(NP_F8)[: KW * 2 * P]
    g8 = np.ascontiguousarray(G.T * SCL_U).astype(np.float32).astype(NP_F8)
    w28 = np.ascontiguousarray(np.asarray(W2, f64).T * SCL_S).astype(np.float32).astype(NP_F8)

    # [P, 16] partition-major consts: col j = cu[j*?]... feature f = k*128+p
    cu_pk = np.ascontiguousarray(cu.astype(np.float32).reshape(KF, P).T)
    bt_pk = np.ascontiguousarray((btot * SCL_S).astype(np.float32).reshape(KF, P).T)
    consts = np.concatenate([cu_pk, bt_pk], axis=1)

    x = np.concatenate(
        [np.asarray(gnn_features, np.float32), np.asarray(transformer_features, np.float32)],
        axis=1,
    )  # [B, K]
    xh_full = x.astype(NP_F8)
    xl_full = (x - xh_full.astype(np.float32)).astype(NP_F8)

    shared = {
        "wshi": ws_hi,
        "wslo": ws_lo,
        "g8": g8,
        "w28": w28,
        "consts": consts,
    }
    in_maps = []
    for i in range(N_CORES):
        rows = slice(i * B_LOC, (i + 1) * B_LOC)
        in_maps.append(
            {
                "xhi": np.ascontiguousarray(xh_full[rows].T),
                "xlo": np.ascontiguousarray(xl_full[rows].T),
                **shared,
            }
        )
    return in_maps


def run(inputs, trace=False, **kw):
    nc = build_module()
    in_maps = prepare_inputs(**inputs)
    res = run_bass_kernel_spmd(nc, in_maps, core_ids=list(range(N_CORES)), trace=trace, **kw)
    out = np.concatenate(
        [r["outT"].T.astype(np.float32) for r in res.results], axis=0
    ) * (1.0 / SCL_S)
    return out, res


def kernel(**inputs) -> np.ndarray:
    out, _ = run(inputs, trace=False)
    return out
